# revision 1
# baseline (speedup 1.0000x reference)
"""Trainium2 Bass kernel for nn_EquivariantLayer (gnn_message_passing).

Self-contained: host-side edge bucketing/padding by destination window,
8-core SPMD Bass kernel (bf16 indirect-gather + tensor-product matmuls +
one-hot segment-sum on PE), output reassembled to full shape.
"""
import numpy as np
import time
import jax
from jax.sharding import Mesh, PartitionSpec
from jax.experimental.shard_map import shard_map
import concourse.bass as bass
import concourse.mybir as mybir
import concourse.tile as tile
from concourse import bacc
from concourse.bass2jax import _bass_exec_p, install_neuronx_cc_hook, partition_id_tensor

F32 = mybir.dt.float32
BF16 = mybir.dt.bfloat16
I32 = mybir.dt.int32
P = 128
N_NODES = 100000
N_CORES = 8


MUL_S, MUL_V, DIM = 64, 32, 160
ESC = 16
ISQ3 = 1.0 / np.sqrt(3.0)
ISQ2 = 1.0 / np.sqrt(2.0)

# planar permutation: planar col 64+32*d+u  <- interleaved col 64+3*u+d
PERM = np.concatenate(
    [np.arange(64)] + [64 + 3 * np.arange(32) + d for d in range(3)]
).astype(np.int64)


def build_plan(edge_index, N, n_cores, P=128):
    """Partition edges by dst core/window. Returns per-core slot maps."""
    src = np.asarray(edge_index[0], dtype=np.int64)
    dst = np.asarray(edge_index[1], dtype=np.int64)
    E = src.shape[0]
    npc = (N + n_cores - 1) // n_cores          # nodes per core
    W = (npc + P - 1) // P                      # windows per core
    core_of = dst // npc
    lw = (dst % npc) // P                       # window within core
    slot = (dst % npc) % P                      # node slot within window
    # count edges per (core, window)
    cw = core_of * W + lw
    counts = np.bincount(cw, minlength=n_cores * W).reshape(n_cores, W)
    G = int(np.ceil(counts.max() / P))
    # order edges by (core, window)
    order = np.argsort(cw, kind="stable")
    return dict(src=src, dst=dst, order=order, counts=counts, G=G, W=W,
                npc=npc, P=P, n_cores=n_cores, E=E,
                slot=slot, core_of=core_of, lw=lw)


def build_core_arrays(plan, c, edge_feat, edge_scalars):
    """Dense padded per-core arrays in (w, p, g) slot layout."""
    P, W, G = plan["P"], plan["W"], plan["G"]
    src, order, counts = plan["src"], plan["order"], plan["counts"]
    slot_all = plan["slot"]
    npc = plan["npc"]
    Wslots = W * G * P

    idx = np.zeros((W, P, G), dtype=np.int32)
    dstw = np.zeros((W, P, G), dtype=np.float32)
    ef = np.zeros((W, P, G, 4), dtype=np.float32)
    escT = np.zeros((W, 17, G * P), dtype=np.float32)
    escT[:, 16, :] = 1.0

    # edges of this core, ordered by window
    base = np.searchsorted(np.sort(plan["core_of"][order]), c, side="left")
    # simpler: recompute masks per core (E=800k, fine)
    cw_sorted_edges = order[(plan["core_of"][order] == c)]
    # cw_sorted_edges are ordered by window (stable argsort of core*W+lw)
    off = 0
    for w in range(W):
        n_e = counts[c, w]
        e_ids = cw_sorted_edges[off:off + n_e]
        off += n_e
        # place edge j at (g = j // P, p = j % P)
        j = np.arange(n_e)
        g, p = j // P, j % P
        idx[w, p, g] = src[e_ids]
        dstw[w, p, g] = slot_all[e_ids]
        ef[w, p, g, 0] = edge_feat[e_ids, 0]
        ef[w, p, g, 1:4] = edge_feat[e_ids, 1:4]
        escT[w, :16, g * P + p] = edge_scalars[e_ids]
    return dict(idx=idx, dstw=dstw, ef=ef, escT=escT)


def prep_weights(W_ss, W_sv, W_vs, W_vv, W_rad, b_rad, L_s, L_v):
    c_ss = ISQ2 / np.sqrt(MUL_S)
    c_vv = ISQ2 * ISQ3 / np.sqrt(MUL_V)
    c_sv = ISQ2 * ISQ3 / np.sqrt(MUL_S)
    c_vs = ISQ2 * ISQ3 / np.sqrt(MUL_V)
    Wss = (W_ss * c_ss).astype(np.float32)            # [64,64]
    Wvv = (W_vv * c_vv).astype(np.float32)            # [32,64]
    Wsv = (W_sv * c_sv).astype(np.float32)            # [64,32]
    Wvs = (W_vs * c_vs).astype(np.float32)            # [32,32]
    Wrs = np.zeros((17, DIM), dtype=np.float32)       # radial, planar cols
    Wrs[:16] = W_rad.T[:, PERM]
    Wrs[16] = b_rad[PERM]
    Ls = (L_s / np.sqrt(MUL_S)).astype(np.float32)
    Lv = (L_v / np.sqrt(MUL_V)).astype(np.float32)
    return dict(Wss=Wss, Wvv=Wvv, Wsv=Wsv, Wvs=Wvs, Wrs=Wrs, Ls=Ls, Lv=Lv)


def ref_core_numpy(xp, core_arrays, wts, xw, P=128):
    """Numpy forward of the kernel math for one core (planar domain).
    xp: [N,160] planar node features. xw: [W,128,160] window residual rows.
    Returns y [W,128,160] planar."""
    idx, dstw, ef, escT = (core_arrays[k] for k in ("idx", "dstw", "ef", "escT"))
    W, _, G = idx.shape
    y = np.zeros((W, P, DIM), dtype=np.float32)
    for w in range(W):
        agg = np.zeros((P, DIM), dtype=np.float32)
        for g in range(G):
            xe = xp[idx[w, :, g]]                      # [128,160]
            es = ef[w, :, g, 0:1]
            ev = ef[w, :, g, 1:4]                      # [128,3]
            xs = xe[:, :64]
            xv = [xe[:, 64 + 32 * d:96 + 32 * d] for d in range(3)]
            out_s = (es * xs) @ wts["Wss"]
            for d in range(3):
                out_s += (ev[:, d:d + 1] * xv[d]) @ wts["Wvv"]
            msg = np.zeros((P, DIM), dtype=np.float32)
            msg[:, :64] = out_s
            for d in range(3):
                msg[:, 64 + 32 * d:96 + 32 * d] = (
                    (ev[:, d:d + 1] * xs) @ wts["Wsv"]
                    + (es * xv[d]) @ wts["Wvs"])
            r = escT[w, :, g * P:(g + 1) * P].T @ wts["Wrs"]   # [128,160]
            msg = msg / (1 + np.exp(-r))
            S = (np.arange(P)[None, :] == dstw[w, :, g][:, None]).astype(np.float32)
            agg += S.T @ msg
        h = xw[w] + agg
        y[w, :, :64] = h[:, :64] @ wts["Ls"]
        for d in range(3):
            y[w, :, 64 + 32 * d:96 + 32 * d] = h[:, 64 + 32 * d:96 + 32 * d] @ wts["Lv"]
    return y


def assemble_output(y_cores, plan, N):
    """y_cores: list of [W,128,160] planar per core -> full [N,160] interleaved."""
    P, W, npc = plan["P"], plan["W"], plan["npc"]
    out = np.zeros((N, DIM), dtype=np.float32)
    inv = np.argsort(PERM)
    for c, yc in enumerate(y_cores):
        flat = yc.reshape(W * P, DIM)[:npc]
        n0 = c * npc
        n1 = min(N, n0 + npc)
        out[n0:n1] = flat[:n1 - n0][:, inv]
    return out


def make_xw(xp, plan, c):
    P, W, npc = plan["P"], plan["W"], plan["npc"]
    xw = np.zeros((W * P, DIM), dtype=np.float32)
    n0 = c * npc
    n1 = min(xp.shape[0], n0 + npc)
    xw[:n1 - n0] = xp[n0:n1]
    return xw.reshape(W, P, DIM)




def build_nc(N, W, G, mode=0):
    """One SPMD program; per-core data differs, program identical."""
    nc = bacc.Bacc(None, target_bir_lowering=False)
    EW = G * P  # edges per window

    xg = nc.declare_dram_parameter("xg", [N, DIM], BF16, isOutput=False)
    idx = nc.declare_dram_parameter("idx", [W, P, G], I32, isOutput=False)
    dstw = nc.declare_dram_parameter("dstw", [W, P, G], F32, isOutput=False)
    ef = nc.declare_dram_parameter("ef", [W, P, G * 4], F32, isOutput=False)
    escT = nc.declare_dram_parameter("escT", [W, 17, EW], F32, isOutput=False)
    xw = nc.declare_dram_parameter("xw", [W, P, DIM], F32, isOutput=False)
    Wss = nc.declare_dram_parameter("Wss", [64, 64], F32, isOutput=False)
    Wvv = nc.declare_dram_parameter("Wvv", [32, 64], F32, isOutput=False)
    Wsv = nc.declare_dram_parameter("Wsv", [64, 32], F32, isOutput=False)
    Wvs = nc.declare_dram_parameter("Wvs", [32, 32], F32, isOutput=False)
    Wrs = nc.declare_dram_parameter("Wrs", [17, DIM], F32, isOutput=False)
    Ls = nc.declare_dram_parameter("Ls", [64, 64], F32, isOutput=False)
    Lv = nc.declare_dram_parameter("Lv", [32, 32], F32, isOutput=False)
    iota = nc.declare_dram_parameter("iota", [P, P], F32, isOutput=False)
    ident = nc.declare_dram_parameter("ident", [P, P], F32, isOutput=False)
    Y = nc.declare_dram_parameter("y", [W, P, DIM], F32, isOutput=True)

    with tile.TileContext(nc) as tc:
        with (
            tc.tile_pool(name="const", bufs=1) as cpool,
            tc.tile_pool(name="win", bufs=3) as wpool,
            tc.tile_pool(name="grp", bufs=4) as gpool,
            tc.tile_pool(name="ps", bufs=1, space="PSUM") as pspool,
            tc.tile_pool(name="ps2", bufs=2, space="PSUM") as ps2pool,
            tc.tile_pool(name="psagg", bufs=1, space="PSUM") as paggpool,
        ):
            # constants
            c_iota = cpool.tile([P, P], F32, tag="iota")
            c_id = cpool.tile([P, P], F32, tag="ident")
            c_wss = cpool.tile([64, 64], F32, tag="wss")
            c_wvv = cpool.tile([32, 64], F32, tag="wvv")
            c_wsv = cpool.tile([64, 32], F32, tag="wsv")
            c_wvs = cpool.tile([32, 32], F32, tag="wvs")
            c_wrs = cpool.tile([17, DIM], F32, tag="wrs")
            c_ls = cpool.tile([64, 64], F32, tag="ls")
            c_lv = cpool.tile([32, 32], F32, tag="lv")
            for t, d in ((c_iota, iota), (c_id, ident), (c_wss, Wss),
                         (c_wvv, Wvv), (c_wsv, Wsv), (c_wvs, Wvs),
                         (c_wrs, Wrs), (c_ls, Ls), (c_lv, Lv)):
                nc.sync.dma_start(out=t[:], in_=d[:])

            for w in range(W):
                t_idx = wpool.tile([P, G], I32, tag="idx")
                t_dstw = wpool.tile([P, G], F32, tag="dstw")
                t_ef = wpool.tile([P, G * 4], F32, tag="ef")
                t_escT = wpool.tile([17, EW], F32, tag="escT")
                t_xw = wpool.tile([P, DIM], F32, tag="xw")
                t_xe = wpool.tile([P, G * DIM], BF16, tag="xe")
                nc.sync.dma_start(out=t_idx[:], in_=idx[w])
                nc.sync.dma_start(out=t_dstw[:], in_=dstw[w])
                nc.sync.dma_start(out=t_ef[:], in_=ef[w])
                nc.sync.dma_start(out=t_escT[:], in_=escT[w])
                nc.sync.dma_start(out=t_xw[:], in_=xw[w])
                for g in range(G):
                    nc.gpsimd.indirect_dma_start(
                        out=t_xe[:, g * DIM:(g + 1) * DIM], out_offset=None,
                        in_=xg[:, :],
                        in_offset=bass.IndirectOffsetOnAxis(ap=t_idx[:, g:g + 1], axis=0))

                t_agg = wpool.tile([P, DIM], F32, tag="agg")
                for g in range(G):
                    xe_g = t_xe[:, g * DIM:(g + 1) * DIM]
                    es = t_ef[:, g * 4:g * 4 + 1]
                    # radial -> sigmoid gate
                    p_r = ps2pool.tile([P, DIM], F32, tag="pr", space="PSUM")
                    nc.tensor.matmul(out=p_r[:], lhsT=t_escT[:, g * P:(g + 1) * P],
                                     rhs=c_wrs[:], start=True, stop=True)
                    t_scale = gpool.tile([P, DIM], F32, tag="scale")
                    nc.scalar.activation(out=t_scale[:], in_=p_r[:],
                                         func=mybir.ActivationFunctionType.Sigmoid)
                    # scaled edge-major operands
                    t_af = gpool.tile([P, DIM], F32, tag="af")
                    nc.vector.tensor_scalar(out=t_af[:], in0=xe_g, scalar1=es,
                                            scalar2=None, op0=mybir.AluOpType.mult)
                    t_gd = [gpool.tile([P, 96], F32, tag=f"gd{d}", name=f"gd{d}") for d in range(3)]
                    for d in range(3):
                        evd = t_ef[:, g * 4 + 1 + d:g * 4 + 2 + d]
                        nc.vector.tensor_scalar(out=t_gd[d][:, 0:64], in0=xe_g[:, 0:64],
                                                scalar1=evd, scalar2=None,
                                                op0=mybir.AluOpType.mult)
                        nc.vector.tensor_scalar(out=t_gd[d][:, 64:96],
                                                in0=xe_g[:, 64 + 32 * d:96 + 32 * d],
                                                scalar1=evd, scalar2=None,
                                                op0=mybir.AluOpType.mult)
                    # transposes (PE) + evac (DVE)
                    p_tA = pspool.tile([64, 512], F32, tag="ptA", space="PSUM")
                    nc.tensor.transpose(out=p_tA[:, 0:128], in_=t_af[:, 0:64], identity=c_id[:])
                    for d in range(3):
                        nc.tensor.transpose(out=p_tA[0:32, 128 + 128 * d:256 + 128 * d],
                                            in_=t_af[:, 64 + 32 * d:96 + 32 * d],
                                            identity=c_id[:])
                    t_aT = gpool.tile([64, 512], F32, tag="aT")
                    nc.vector.tensor_copy(out=t_aT[:, 0:128], in_=p_tA[:, 0:128])
                    nc.vector.tensor_copy(out=t_aT[0:32, 128:512], in_=p_tA[0:32, 128:512])
                    p_tG = pspool.tile([64, 384], F32, tag="ptG", space="PSUM")
                    p_tG2 = pspool.tile([32, 384], F32, tag="ptG2", space="PSUM")
                    for d in range(3):
                        nc.tensor.transpose(out=p_tG[:, 128 * d:128 * d + 128],
                                            in_=t_gd[d][:, 0:64], identity=c_id[:])
                        nc.tensor.transpose(out=p_tG2[:, 128 * d:128 * d + 128],
                                            in_=t_gd[d][:, 64:96], identity=c_id[:])
                    t_gT = gpool.tile([64, 384], F32, tag="gT")
                    t_gT2 = gpool.tile([32, 384], F32, tag="gT2")
                    nc.vector.tensor_copy(out=t_gT[:], in_=p_tG[:])
                    nc.vector.tensor_copy(out=t_gT2[:], in_=p_tG2[:])
                    # TP matmuls into p_tp
                    t_msg = gpool.tile([P, DIM], F32, tag="msg")
                    if not (mode & 1):
                        p_tp = ps2pool.tile([P, DIM], F32, tag="ptp", space="PSUM")
                        nc.tensor.matmul(out=p_tp[:, 0:64], lhsT=t_aT[:, 0:128],
                                         rhs=c_wss[:], start=True, stop=False)
                        for d in range(3):
                            nc.tensor.matmul(out=p_tp[:, 0:64],
                                             lhsT=t_gT2[:, 128 * d:128 * d + 128],
                                             rhs=c_wvv[:], start=False, stop=(d == 2))
                        for d in range(3):
                            sl = slice(64 + 32 * d, 96 + 32 * d)
                            nc.tensor.matmul(out=p_tp[:, sl],
                                             lhsT=t_gT[:, 128 * d:128 * d + 128],
                                             rhs=c_wsv[:], start=True, stop=False)
                            nc.tensor.matmul(out=p_tp[:, sl],
                                             lhsT=t_aT[0:32, 128 + 128 * d:256 + 128 * d],
                                             rhs=c_wvs[:], start=False, stop=True)
                        nc.vector.tensor_tensor(out=t_msg[:], in0=p_tp[:], in1=t_scale[:],
                                                op=mybir.AluOpType.mult)
                    else:
                        nc.vector.tensor_tensor(out=t_msg[:], in0=t_af[:], in1=t_scale[:],
                                                op=mybir.AluOpType.mult)
                    t_S = gpool.tile([P, P], F32, tag="S")
                    nc.vector.tensor_scalar(out=t_S[:], in0=c_iota[:],
                                            scalar1=t_dstw[:, g:g + 1], scalar2=None,
                                            op0=mybir.AluOpType.is_equal)
                    p_a = paggpool.tile([P, DIM], F32, tag="pagg", space="PSUM")
                    nc.tensor.matmul(out=p_a[:], lhsT=t_S[:], rhs=t_msg[:],
                                     start=True, stop=True)
                    if g == 0:
                        nc.vector.tensor_copy(out=t_agg[:], in_=p_a[:])
                    else:
                        nc.vector.tensor_tensor(out=t_agg[:], in0=t_agg[:], in1=p_a[:],
                                                op=mybir.AluOpType.add)

                # window epilogue: h = xw + agg ; y = h @ L (per irrep)
                t_h = wpool.tile([P, DIM], F32, tag="h")
                nc.vector.tensor_tensor(out=t_h[:], in0=t_agg[:], in1=t_xw[:],
                                        op=mybir.AluOpType.add)
                p_hT = pspool.tile([64, 512], F32, tag="ptA", space="PSUM")
                nc.tensor.transpose(out=p_hT[:, 0:128], in_=t_h[:, 0:64], identity=c_id[:])
                for d in range(3):
                    nc.tensor.transpose(out=p_hT[0:32, 128 + 128 * d:256 + 128 * d],
                                        in_=t_h[:, 64 + 32 * d:96 + 32 * d],
                                        identity=c_id[:])
                t_hT = wpool.tile([64, 512], F32, tag="hT")
                nc.vector.tensor_copy(out=t_hT[:, 0:128], in_=p_hT[:, 0:128])
                nc.vector.tensor_copy(out=t_hT[0:32, 128:512], in_=p_hT[0:32, 128:512])
                p_y = ps2pool.tile([P, DIM], F32, tag="ptp", space="PSUM")
                nc.tensor.matmul(out=p_y[:, 0:64], lhsT=t_hT[:, 0:128], rhs=c_ls[:],
                                 start=True, stop=True)
                for d in range(3):
                    sl = slice(64 + 32 * d, 96 + 32 * d)
                    nc.tensor.matmul(out=p_y[:, sl],
                                     lhsT=t_hT[0:32, 128 + 128 * d:256 + 128 * d],
                                     rhs=c_lv[:], start=True, stop=True)
                t_y = wpool.tile([P, DIM], F32, tag="y")
                nc.vector.tensor_copy(out=t_y[:], in_=p_y[:])
                nc.sync.dma_start(out=Y[w], in_=t_y[:])
    nc.compile()
    return nc



class SpmdRunner:
    def __init__(self, nc, n_cores=8):
        install_neuronx_cc_hook()
        self.nc = nc
        self.n_cores = n_cores
        assert nc.dbg_addr is None or not nc.dbg_callbacks
        partition_name = nc.partition_id_tensor.name if nc.partition_id_tensor else None
        in_names, out_names, out_avals, zero_outs = [], [], [], []
        for alloc in nc.m.functions[0].allocations:
            if not isinstance(alloc, mybir.MemoryLocationSet):
                continue
            name = alloc.memorylocations[0].name
            if alloc.kind == "ExternalInput":
                if name != partition_name:
                    in_names.append(name)
            elif alloc.kind == "ExternalOutput":
                shape = tuple(alloc.tensor_shape)
                dtype = mybir.dt.np(alloc.dtype)
                out_names.append(name)
                out_avals.append(jax.core.ShapedArray(shape, dtype))
                zero_outs.append(np.zeros(shape, dtype))
        self.in_names, self.out_names = in_names, out_names
        self.out_avals, self.zero_outs = out_avals, zero_outs
        n_params, n_outs = len(in_names), len(out_names)
        self.n_params = n_params
        all_in_names = list(in_names) + list(out_names)
        if partition_name is not None:
            all_in_names.append(partition_name)

        def _body(*args):
            operands = list(args)
            if partition_name is not None:
                operands.append(partition_id_tensor())
            outs = _bass_exec_p.bind(
                *operands,
                out_avals=tuple(out_avals),
                in_names=tuple(all_in_names),
                out_names=tuple(out_names),
                lowering_input_output_aliases=(),
                sim_require_finite=False,
                sim_require_nnan=False,
                nc=nc,
            )
            return tuple(outs)

        devices = jax.devices()[:n_cores]
        self.mesh = Mesh(np.asarray(devices), ("core",))
        in_specs = (PartitionSpec("core"),) * (n_params + n_outs)
        out_specs = (PartitionSpec("core"),) * n_outs
        # NO donation: lets us reuse the same zero-buffers across timed calls.
        self.fn = jax.jit(
            shard_map(_body, mesh=self.mesh, in_specs=in_specs, out_specs=out_specs,
                      check_rep=False),
            keep_unused=True,
        )
        self._concat_cache = None

    def prepare(self, in_maps):
        """Concat per-core inputs and device_put once."""
        per_core = [[np.asarray(in_maps[c][n]) for n in self.in_names]
                    for c in range(self.n_cores)]
        concat_in = [np.concatenate([per_core[c][i] for c in range(self.n_cores)], axis=0)
                     for i in range(self.n_params)]
        concat_zero = [np.zeros((self.n_cores * z.shape[0], *z.shape[1:]), z.dtype)
                       for z in self.zero_outs]
        args = concat_in + concat_zero
        sh = jax.sharding.NamedSharding(self.mesh, PartitionSpec("core"))
        self._args = [jax.device_put(a, sh) for a in args]
        return self._args

    def run(self):
        outs = self.fn(*self._args)
        jax.block_until_ready(outs)
        return outs

    def results(self, outs):
        res = []
        for c in range(self.n_cores):
            d = {}
            for i, name in enumerate(self.out_names):
                d[name] = np.asarray(outs[i]).reshape(self.n_cores, *self.out_avals[i].shape)[c]
            res.append(d)
        return res

    def time(self, iters=10, warmup=2):
        for _ in range(warmup):
            self.run()
        ts = []
        for _ in range(iters):
            t0 = time.perf_counter()
            self.run()
            ts.append(time.perf_counter() - t0)
        return np.array(ts)


_CACHE = {}


def kernel(x, edge_index, edge_feat, edge_scalars,
           W_ss, W_sv, W_vs, W_vv, W_rad, b_rad, L_s, L_v):
    x = np.asarray(x, dtype=np.float32)
    edge_index = np.asarray(edge_index)
    edge_feat = np.asarray(edge_feat, dtype=np.float32)
    edge_scalars = np.asarray(edge_scalars, dtype=np.float32)
    N = x.shape[0]

    xp = np.ascontiguousarray(x[:, PERM])
    xg_bf16 = xp.astype(mybir.dt.np(BF16))
    plan = build_plan(edge_index, N, N_CORES)
    G, W = plan["G"], plan["W"]
    wts = prep_weights(np.asarray(W_ss), np.asarray(W_sv), np.asarray(W_vs),
                       np.asarray(W_vv), np.asarray(W_rad), np.asarray(b_rad),
                       np.asarray(L_s), np.asarray(L_v))
    in_maps = []
    for c in range(N_CORES):
        ca = build_core_arrays(plan, c, edge_feat, edge_scalars)
        xwc = make_xw(xp, plan, c)
        in_maps.append(dict(
            xg=xg_bf16, idx=ca["idx"], dstw=ca["dstw"],
            ef=ca["ef"].reshape(W, 128, G * 4), escT=ca["escT"], xw=xwc,
            Wss=wts["Wss"], Wvv=wts["Wvv"], Wsv=wts["Wsv"], Wvs=wts["Wvs"],
            Wrs=wts["Wrs"], Ls=wts["Ls"], Lv=wts["Lv"],
            iota=np.tile(np.arange(128, dtype=np.float32)[None, :], (128, 1)),
            ident=np.eye(128, dtype=np.float32)))

    key = (N, W, G)
    if key not in _CACHE:
        nc = build_nc(N, W, G)
        _CACHE[key] = SpmdRunner(nc, n_cores=N_CORES)
    runner = _CACHE[key]
    runner.prepare(in_maps)
    outs = runner.run()
    res = runner.results(outs)
    out = assemble_output([res[c]["y"] for c in range(N_CORES)], plan, N)
    return out.astype(np.float32)

